# revision 17
# baseline (speedup 1.0000x reference)
"""Contrastive loss (topk_masking) Trainium2 Bass kernel — v2.

Math: reference computes, for each direction (t2i and i2t),
    d = txt @ img.T                      # [B,B]
    pos = diag(d)
    negs = top-128 of each row of d (diag masked to 0)
    loss_row = logsumexp([pos, negs + margin] / lamda) - pos/lamda
    loss = mean(loss_row);  final = 0.5*(t2i + i2t)

With lamda = 0.01 the logsumexp over the top-128 row values equals (to f32
precision) the logsumexp over ALL off-diagonal row values, so the kernel
computes full-row streaming max + sum-exp instead of a top-k.

v2 design (per-engine work minimization):
  - txt is pre-scaled by 100 (=1/lamda) on host, so PSUM holds 100*d and
    no separate bias-prep ops are needed.
  - Per 1024-column chunk: Vector reduce_max(negate=True) -> negmx (SBUF),
    Scalar activation(Exp, bias=negmx, accum_out=S4) does exp + row-sum in
    ONE pass, writing the (unused) exp values to a bf16 SBUF scratch so the
    PSUM bank frees early.  No mask multiply: the diagonal's contribution
    exp(pos100 + 20 - Bref) is subtracted analytically on the host.
  - Matmuls are chunk-local so the hoisted semaphore waits (bacc moves
    matmul waits onto the preceding LDWEIGHTS) gate only on that chunk's
    PSUM banks being free, keeping PE stalls short and local.
  - All input DMA rides the Scalar-engine hardware DGE queue, which
    sustains ~220 GB/s on this setup (the Sync queue only ~20-80 GB/s and
    starves the PE); pieces are issued in matmul consumption order.
  - Device outputs only per-chunk stats (negmx, S4) [128, 32] f32 each;
    the whole epilogue (Bref, rescale, ln, mean) runs on host in f64,
    including pos (row-wise dot) -- removes ime/tme loads, the Ln table
    load and the serial cross-engine tail.

Sharding: each core i owns 512 rows of each direction; inputs are rotated
(rows rolled by i*512) so the program is SPMD: row block = rows 0..511,
diagonal in columns 0..511.  Host pre-transposes (D on partitions) and
casts to bf16.
"""

import numpy as np
import ml_dtypes

B = 4096
D = 256
NCORES = 8
RPC = B // NCORES          # 512 rows per core
G = RPC // 128             # 4 partition-groups of 128 rows
NCH = 4                    # column chunks per row-group
CW = B // NCH              # 1024 columns per chunk (2 PSUM banks)
N_MM = 512                 # matmul moving free dim
NK = 2 * G * NCH           # stat columns per core (dir,group,chunk)
LAMDA = 0.01
MARGIN = 0.2
MARGIN_S = MARGIN / LAMDA  # 20.0

_CACHE = {}


def _build_nc():
    import concourse.bacc as bacc
    import concourse.tile as tile
    from concourse import mybir

    f32 = mybir.dt.float32
    bf16 = mybir.dt.bfloat16
    AX = mybir.AxisListType.X
    OP = mybir.AluOpType
    AF = mybir.ActivationFunctionType

    nc = bacc.Bacc(
        "TRN2",
        target_bir_lowering=False,
        debug=False,
        num_devices=NCORES,
    )

    imgT_d = nc.dram_tensor("imgT", (D, B), bf16, kind="ExternalInput")
    txtT_d = nc.dram_tensor("txtT", (D, B), bf16, kind="ExternalInput")
    negmx_d = nc.dram_tensor("negmx", (128, NK), f32, kind="ExternalOutput")
    s4_d = nc.dram_tensor("s4", (128, NK), f32, kind="ExternalOutput")

    with tile.TileContext(nc) as tc:
        with (
            tc.tile_pool(name="big", bufs=1) as big,
            tc.tile_pool(name="small", bufs=1) as small,
            tc.tile_pool(name="scr", bufs=2) as scr,
            tc.tile_pool(name="psum", bufs=1, space="PSUM") as pp,
        ):
            # ---- persistent loads (D on partitions; two 128-halves of D) ----
            # dir-0 needs txtT[:, 0:512] (weights) + all of imgT (moving);
            # dir-1 needs the rest of txtT.  Load in consumption order,
            # split across BOTH hardware DGE queues (sync + scalar) in
            # 2048-col pieces (4 KiB per-partition runs) — a single queue
            # sustains only ~80 GB/s and starves the PE.
            imgT = [big.tile([128, B], bf16, tag=f"imgT{h}", name=f"imgT{h}") for h in range(2)]
            txtT = [big.tile([128, B], bf16, tag=f"txtT{h}", name=f"txtT{h}") for h in range(2)]
            # ALL input loads go through the Scalar HW DGE queue: it sustains
            # ~220 GB/s (vs ~20-80 GB/s for the Sync queue, which would
            # starve the PE).  Pieces are issued in consumption order; the
            # queue streams them back-to-back, so everything is resident by
            # ~29 us while compute runs to ~60 us.
            for h in range(2):
                nc.scalar.dma_start(
                    txtT[h][:, 0:1024], txtT_d[h * 128:(h + 1) * 128, 0:1024])
                nc.scalar.dma_start(
                    imgT[h][:, 0:1024], imgT_d[h * 128:(h + 1) * 128, 0:1024])
            for h in range(2):
                nc.scalar.dma_start(
                    imgT[h][:, 1024:B], imgT_d[h * 128:(h + 1) * 128, 1024:B])
            for h in range(2):
                nc.scalar.dma_start(
                    txtT[h][:, 1024:B], txtT_d[h * 128:(h + 1) * 128, 1024:B])

            negmx = small.tile([128, NK], f32, tag="negmx")
            s4 = small.tile([128, NK], f32, tag="s4")

            # ---- main: for each direction and row-group, stream chunks ----
            for di, (lh, rh) in enumerate(((txtT, imgT), (imgT, txtT))):
                for g in range(G):
                    col = di * G + g
                    pcs = [pp.tile([128, CW], f32, tag=f"pc{c}", name=f"pc{c}") for c in range(NCH)]
                    # chunk-local matmuls (h-outer within the chunk): the
                    # hoisted semaphore waits land on the chunk's first
                    # LDWEIGHTS, so the PE only ever stalls on THIS chunk's
                    # PSUM being free, not on the whole group.
                    for c in range(NCH):
                        for h in range(2):
                            w = lh[h][:, g * 128:(g + 1) * 128]
                            for s in range(0, CW, N_MM):
                                nc.tensor.matmul(
                                    pcs[c][:, s:s + N_MM], w,
                                    rh[h][:, c * CW + s:c * CW + s + N_MM],
                                    start=(h == 0), stop=(h == 1))
                        k = col * NCH + c
                        nc.vector.reduce_max(
                            negmx[:, k:k + 1], pcs[c][:], AX, negate=True)
                        eout = scr.tile([128, CW], bf16, tag="eout")
                        nc.scalar.activation(
                            eout[:], pcs[c][:], AF.Exp,
                            bias=negmx[:, k:k + 1], scale=1.0,
                            accum_out=s4[:, k:k + 1])
                # stats for this direction are complete -> ship them.
                # dir-0 on sync (engine idle, deps satisfied mid-kernel);
                # dir-1 on the fast scalar queue: its two issue slots run
                # after the last activation, and the slow sync queue would
                # otherwise gate the final barrier by ~1.2 us.
                lo, hi = di * G * NCH, (di + 1) * G * NCH
                eng = nc.sync if di == 0 else nc.scalar
                eng.dma_start(negmx_d[:, lo:hi], negmx[:, lo:hi])
                eng.dma_start(s4_d[:, lo:hi], s4[:, lo:hi])

    nc.compile()
    return nc


def get_nc():
    if "nc" not in _CACHE:
        _CACHE["nc"] = _build_nc()
    return _CACHE["nc"]


def make_in_maps(img, txt):
    """Host-side shard prep: rotate rows per core, transpose, cast to bf16.

    txt is additionally scaled by 1/lamda = 100 so PSUM holds 100*d.
    """
    bf = ml_dtypes.bfloat16
    img = np.ascontiguousarray(np.asarray(img, dtype=np.float32))
    txt = np.asarray(txt, dtype=np.float32) * 100.0
    imgT2 = np.concatenate([img.T, img.T], axis=1).astype(bf)   # [D, 2B]
    txtT2 = np.concatenate([txt.T, txt.T], axis=1).astype(bf)
    in_maps = []
    for i in range(NCORES):
        r0 = i * RPC
        in_maps.append({
            "imgT": np.ascontiguousarray(imgT2[:, r0:r0 + B]),
            "txtT": np.ascontiguousarray(txtT2[:, r0:r0 + B]),
        })
    return in_maps


def run_device(nc, in_maps, **kwargs):
    from concourse.bass_utils import run_bass_kernel_spmd
    return run_bass_kernel_spmd(nc, in_maps, core_ids=list(range(NCORES)), **kwargs)


def kernel(img, txt, txt_lens=None, **_ignored):
    nc = get_nc()
    img = np.ascontiguousarray(np.asarray(img, dtype=np.float32))
    txt = np.ascontiguousarray(np.asarray(txt, dtype=np.float32))
    in_maps = make_in_maps(img, txt)
    res = run_device(nc, in_maps)

    # host epilogue in f64
    pos100 = 100.0 * np.einsum(
        'ij,ij->i', txt.astype(np.float64), img.astype(np.float64))  # [B]
    total = 0.0
    for i, r in enumerate(res.results):
        r0 = i * RPC
        mx = -np.asarray(r["negmx"], dtype=np.float64)   # [128, NK]
        s4 = np.asarray(r["s4"], dtype=np.float64)       # [128, NK]
        # k = (di*G + g)*NCH + c ; partition p -> local row g*128+p
        mx = mx.reshape(128, 2, G, NCH)
        s4 = s4.reshape(128, 2, G, NCH)
        p100 = pos100[r0 + np.arange(G * 128)].reshape(G, 128).T  # [128, G]
        p100 = p100[:, None, :]                                   # [128,1,G]
        m_row = mx.max(axis=3)                                    # [128,2,G]
        Bref = np.maximum(m_row + MARGIN_S, p100)
        S = (s4 * np.exp(mx + MARGIN_S - Bref[..., None])).sum(axis=3)
        pose = np.exp(p100 - Bref)
        S = S - np.exp(p100 + MARGIN_S - Bref) + pose
        S = np.maximum(S, pose)
        total += (Bref - p100 + np.log(S)).sum()
    return np.array(total / (2.0 * B), dtype=np.float32)


# revision 19
# speedup vs baseline: 1.0208x; 1.0208x over previous
"""Contrastive loss (topk_masking) Trainium2 Bass kernel — v2.

Math: reference computes, for each direction (t2i and i2t),
    d = txt @ img.T                      # [B,B]
    pos = diag(d)
    negs = top-128 of each row of d (diag masked to 0)
    loss_row = logsumexp([pos, negs + margin] / lamda) - pos/lamda
    loss = mean(loss_row);  final = 0.5*(t2i + i2t)

With lamda = 0.01 the logsumexp over the top-128 row values equals (to f32
precision) the logsumexp over ALL off-diagonal row values, so the kernel
computes full-row streaming max + sum-exp instead of a top-k.

v2 design (per-engine work minimization):
  - txt is pre-scaled by 100 (=1/lamda) on host, so PSUM holds 100*d and
    no separate bias-prep ops are needed.
  - Per 1024-column chunk: Vector reduce_max(negate=True) -> negmx (SBUF),
    Scalar activation(Exp, bias=negmx, accum_out=S4) does exp + row-sum in
    ONE pass, writing the (unused) exp values to a bf16 SBUF scratch so the
    PSUM bank frees early.  No mask multiply: the diagonal's contribution
    exp(pos100 + 20 - Bref) is subtracted analytically on the host.
  - Matmuls are chunk-local so the hoisted semaphore waits (bacc moves
    matmul waits onto the preceding LDWEIGHTS) gate only on that chunk's
    PSUM banks being free, keeping PE stalls short and local.
  - All input DMA rides the Scalar-engine hardware DGE queue, which
    sustains ~220 GB/s on this setup (the Sync queue only ~20-80 GB/s and
    starves the PE); pieces are issued in matmul consumption order.
  - Device outputs only per-chunk stats (negmx, S4) [128, 32] f32 each;
    the whole epilogue (Bref, rescale, ln, mean) runs on host in f64,
    including pos (row-wise dot) -- removes ime/tme loads, the Ln table
    load and the serial cross-engine tail.

Sharding: each core i owns 512 rows of each direction; inputs are rotated
(rows rolled by i*512) so the program is SPMD: row block = rows 0..511,
diagonal in columns 0..511.  Host pre-transposes (D on partitions) and
casts to bf16.
"""

import numpy as np
import ml_dtypes

B = 4096
D = 256
NCORES = 8
RPC = B // NCORES          # 512 rows per core
G = RPC // 128             # 4 partition-groups of 128 rows
NCH = 4                    # column chunks per row-group
CW = B // NCH              # 1024 columns per chunk (2 PSUM banks)
N_MM = 512                 # matmul moving free dim
NK = 2 * G * NCH           # stat columns per core (dir,group,chunk)
LAMDA = 0.01
MARGIN = 0.2
MARGIN_S = MARGIN / LAMDA  # 20.0

_CACHE = {}


def _build_nc():
    import concourse.bacc as bacc
    import concourse.tile as tile
    from concourse import mybir

    f32 = mybir.dt.float32
    bf16 = mybir.dt.bfloat16
    AX = mybir.AxisListType.X
    OP = mybir.AluOpType
    AF = mybir.ActivationFunctionType

    nc = bacc.Bacc(
        "TRN2",
        target_bir_lowering=False,
        debug=False,
        num_devices=NCORES,
    )

    imgT_d = nc.dram_tensor("imgT", (D, B), bf16, kind="ExternalInput")
    txtT_d = nc.dram_tensor("txtT", (D, B), bf16, kind="ExternalInput")
    negmx_d = nc.dram_tensor("negmx", (128, NK), f32, kind="ExternalOutput")
    s4_d = nc.dram_tensor("s4", (128, NK), f32, kind="ExternalOutput")

    with tile.TileContext(nc) as tc:
        with (
            tc.tile_pool(name="big", bufs=1) as big,
            tc.tile_pool(name="small", bufs=1) as small,
            tc.tile_pool(name="scr", bufs=2) as scr,
            tc.tile_pool(name="psum", bufs=1, space="PSUM") as pp,
        ):
            # ---- persistent loads (D on partitions; two 128-halves of D) ----
            # dir-0 needs txtT[:, 0:512] (weights) + all of imgT (moving);
            # dir-1 needs the rest of txtT.
            imgT = [big.tile([128, B], bf16, tag=f"imgT{h}", name=f"imgT{h}") for h in range(2)]
            txtT = [big.tile([128, B], bf16, tag=f"txtT{h}", name=f"txtT{h}") for h in range(2)]
            # ALL input loads go through the Scalar HW DGE queue: it sustains
            # ~220 GB/s (vs ~20-80 GB/s for the Sync queue, which would
            # starve the PE).  Pieces are issued in consumption order; the
            # queue streams them back-to-back, so everything is resident by
            # ~29 us while compute runs to ~60 us.
            for h in range(2):
                nc.scalar.dma_start(
                    txtT[h][:, 0:1024], txtT_d[h * 128:(h + 1) * 128, 0:1024])
                nc.scalar.dma_start(
                    imgT[h][:, 0:1024], imgT_d[h * 128:(h + 1) * 128, 0:1024])
            for h in range(2):
                nc.scalar.dma_start(
                    imgT[h][:, 1024:B], imgT_d[h * 128:(h + 1) * 128, 1024:B])
            for h in range(2):
                nc.scalar.dma_start(
                    txtT[h][:, 1024:B], txtT_d[h * 128:(h + 1) * 128, 1024:B])

            negmx = small.tile([128, NK], f32, tag="negmx")
            s4 = small.tile([128, NK], f32, tag="s4")

            # ---- main: for each direction and row-group, stream chunks ----
            for di, (lh, rh) in enumerate(((txtT, imgT), (imgT, txtT))):
                for g in range(G):
                    col = di * G + g
                    pcs = [pp.tile([128, CW], f32, tag=f"pc{c}", name=f"pc{c}") for c in range(NCH)]
                    # chunk-local matmuls (h-outer within the chunk): the
                    # hoisted semaphore waits land on the chunk's first
                    # LDWEIGHTS, so the PE only ever stalls on THIS chunk's
                    # PSUM being free, not on the whole group.
                    for c in range(NCH):
                        for h in range(2):
                            w = lh[h][:, g * 128:(g + 1) * 128]
                            for s in range(0, CW, N_MM):
                                nc.tensor.matmul(
                                    pcs[c][:, s:s + N_MM], w,
                                    rh[h][:, c * CW + s:c * CW + s + N_MM],
                                    start=(h == 0), stop=(h == 1))
                        k = col * NCH + c
                        nc.vector.reduce_max(
                            negmx[:, k:k + 1], pcs[c][:], AX, negate=True)
                        eout = scr.tile([128, CW], bf16, tag="eout")
                        nc.scalar.activation(
                            eout[:], pcs[c][:], AF.Exp,
                            bias=negmx[:, k:k + 1], scale=1.0,
                            accum_out=s4[:, k:k + 1])
                # stats for this direction are complete -> ship them (sync
                # queue: small, and keeps the Scalar engine's instruction
                # stream free for activations)
                lo, hi = di * G * NCH, (di + 1) * G * NCH
                nc.sync.dma_start(negmx_d[:, lo:hi], negmx[:, lo:hi])
                nc.sync.dma_start(s4_d[:, lo:hi], s4[:, lo:hi])

    nc.compile()
    return nc


def get_nc():
    if "nc" not in _CACHE:
        _CACHE["nc"] = _build_nc()
    return _CACHE["nc"]


def make_in_maps(img, txt):
    """Host-side shard prep: rotate rows per core, transpose, cast to bf16.

    txt is additionally scaled by 1/lamda = 100 so PSUM holds 100*d.
    """
    bf = ml_dtypes.bfloat16
    img = np.ascontiguousarray(np.asarray(img, dtype=np.float32))
    txt = np.asarray(txt, dtype=np.float32) * 100.0
    imgT2 = np.concatenate([img.T, img.T], axis=1).astype(bf)   # [D, 2B]
    txtT2 = np.concatenate([txt.T, txt.T], axis=1).astype(bf)
    in_maps = []
    for i in range(NCORES):
        r0 = i * RPC
        in_maps.append({
            "imgT": np.ascontiguousarray(imgT2[:, r0:r0 + B]),
            "txtT": np.ascontiguousarray(txtT2[:, r0:r0 + B]),
        })
    return in_maps


def run_device(nc, in_maps, **kwargs):
    from concourse.bass_utils import run_bass_kernel_spmd
    return run_bass_kernel_spmd(nc, in_maps, core_ids=list(range(NCORES)), **kwargs)


def kernel(img, txt, txt_lens=None, **_ignored):
    nc = get_nc()
    img = np.ascontiguousarray(np.asarray(img, dtype=np.float32))
    txt = np.ascontiguousarray(np.asarray(txt, dtype=np.float32))
    in_maps = make_in_maps(img, txt)
    res = run_device(nc, in_maps)

    # host epilogue in f64
    pos100 = 100.0 * np.einsum(
        'ij,ij->i', txt.astype(np.float64), img.astype(np.float64))  # [B]
    total = 0.0
    for i, r in enumerate(res.results):
        r0 = i * RPC
        mx = -np.asarray(r["negmx"], dtype=np.float64)   # [128, NK]
        s4 = np.asarray(r["s4"], dtype=np.float64)       # [128, NK]
        # k = (di*G + g)*NCH + c ; partition p -> local row g*128+p
        mx = mx.reshape(128, 2, G, NCH)
        s4 = s4.reshape(128, 2, G, NCH)
        p100 = pos100[r0 + np.arange(G * 128)].reshape(G, 128).T  # [128, G]
        p100 = p100[:, None, :]                                   # [128,1,G]
        m_row = mx.max(axis=3)                                    # [128,2,G]
        Bref = np.maximum(m_row + MARGIN_S, p100)
        S = (s4 * np.exp(mx + MARGIN_S - Bref[..., None])).sum(axis=3)
        pose = np.exp(p100 - Bref)
        S = S - np.exp(p100 + MARGIN_S - Bref) + pose
        S = np.maximum(S, pose)
        total += (Bref - p100 + np.log(S)).sum()
    return np.array(total / (2.0 * B), dtype=np.float32)
